# revision 17
# baseline (speedup 1.0000x reference)
"""Trainium2 Bass kernel for KDPointToPointLoss (exact 1-NN + MSE).

Math: loss = mean_b mean_{n,d} ||s_n - t_{nn(n)}||^2
           = (1/(B*N*3)) * sum_{b,n} min_m ||s_n - t_m||^2
so only the min distance VALUES are needed (no argmin indices / gather).

Exact norm-window pruning: sort sources and targets by radius (the loss is
permutation invariant). For a source tile (128 radius-adjacent sources) with
radius range [a,b] and a certified upper bound W >= max_n sqrt(min-dist_n),
every nearest neighbor lies among targets with radius in [a-W, b+W]: any
other target m has d2 >= (|t_m|-|s_n|)^2 > W^2 >= min-dist. W comes from a
cheap host scan of k rank-adjacent candidates (valid upper bound; the device
still evaluates every certified candidate exactly). This prunes ~85% of the
distance matrix on random clouds.

Device work = flat list of groups (source tile x 1024 gathered target cols):
K=24 bf16 matmul (hi/lo/lo2 splits of s, t, s2, t2 -> fp32-level accuracy)
into PSUM, then a custom 2-input DVE op (min body + min accumulate,
2 elems/cycle) folds each group to one accumulator column. ScalarE stages
half of each group PSUM->SBUF (DVE may read only one PSUM operand).
Matmuls alternate two row-group weight replicas so LDWEIGHTS overlaps the
other group's in-flight matmul. Host min-combines group columns (fp64).

Sharding: 8 cores; cores 0-3 batch 0, cores 4-7 batch 1, balanced by group
count; the gathered rhs keeps per-core inputs small.
"""

import os
import numpy as np
import ml_dtypes

import concourse.bass as bass
import concourse.bacc as bacc
import concourse.mybir as mybir
from concourse.tile import TileContext
from concourse.bass_utils import run_bass_kernel_spmd

bf16 = ml_dtypes.bfloat16

B, N, M, D = 2, 8192, 8192, 3
N_CORES = 8
CORES_PER_BATCH = N_CORES // B
M_CHUNK = 512
GROUP = 1024                 # columns per DVE fold group (2 PSUM banks)
K = 24
K_CAND = 512                 # host candidate scan width for upper bounds
_BIG = 3.0e38

_DMA_SPLIT = 6               # rhs pieces per replica, spread over DMA queues


# ---------------------------------------------------------------- custom DVE op
_MIN2 = None


def _get_min2_op():
    """MIN2_REDUCE_ANT: out = min(in0, in1); accum = min(s0, min(out)).
    Reads 2 tensor streams at 1 elem/cycle each -> 2x native tensor_reduce."""
    global _MIN2
    if _MIN2 is not None:
        return _MIN2
    import concourse.dve_ops as dve_ops
    from concourse.dve_spec import Spec, Src0, Src1, C0, minn, lower, _has_src1
    from concourse.dve_uop import DveOpSpec

    for op in dve_ops.OPS:
        if op.name == "MIN2_REDUCE_ANT":
            _MIN2 = op
            return op

    def _ref(in0, in1, c0, c1, c2):
        b = np.minimum(in0.astype(np.float32), in1.astype(np.float32))
        acc = np.minimum(
            np.minimum.reduce(b.reshape(b.shape[0], -1), axis=-1, keepdims=True),
            np.asarray(c0, np.float32).reshape(-1, 1))
        return b, acc

    spec = Spec(body=minn(Src0, Src1), accum=minn, accum_init=C0, reference=_ref)
    opcode = dve_ops._CUSTOM_DVE_ROW_BASE + len(dve_ops.OPS)
    sha = {}
    for ver in ("v3", "v4"):
        uops = lower(spec, ver=ver)
        sha[ver] = DveOpSpec(name="MIN2_REDUCE_ANT", opcode=opcode, uops=uops,
                             rd1_en=_has_src1(spec)).sha(ver)
    op = dve_ops.DveOp("MIN2_REDUCE_ANT", spec, subdim=False, uops_sha=sha)
    dve_ops.OPS.append(op)
    dve_ops._SUB_OPCODE_FOR_NAME[op.name] = opcode
    _MIN2 = op
    return op


def _split3(x):
    """fp64 array -> (hi, lo, lo2) bf16 triple with residual ~2^-24."""
    x = x.astype(np.float64)
    h = x.astype(bf16)
    r = x - h.astype(np.float64)
    l = r.astype(bf16)
    r2 = r - l.astype(np.float64)
    l2 = r2.astype(bf16)
    return h, l, l2


# ---------------------------------------------------------------- device kernel
_NC_CACHE = {}


def _build_bass(G):
    """Flat loop over G groups: 2 matmuls -> PSUM [128,1024], ScalarE stages
    the second half to SBUF, custom DVE op folds to acc[:, g]."""
    min2 = _get_min2_op()
    nc = bacc.Bacc(trn_type="TRN2")
    # only the 24 real rows ship; both row-group replicas are DMA'd from them
    lhs_d = nc.dram_tensor("lhs", [K, G * 128], mybir.dt.bfloat16, kind="ExternalInput")
    rhs_d = nc.dram_tensor("rhs", [K, G * GROUP], mybir.dt.bfloat16, kind="ExternalInput")
    out_d = nc.dram_tensor("out", [128, G], mybir.dt.float32, kind="ExternalOutput")

    fp32 = mybir.dt.float32

    with TileContext(nc) as tc:
        with (
            tc.tile_pool(name="const", bufs=1) as cpool,
            tc.tile_pool(name="psum", bufs=4, space="PSUM") as ppool,
            tc.tile_pool(name="scratch", bufs=4) as spool,
        ):
            lhs_sb = cpool.tile([64, G * 128], mybir.dt.bfloat16)
            rhs_sb = cpool.tile([64, G * GROUP], mybir.dt.bfloat16)
            acc = cpool.tile([128, G], fp32)

            # two DMA queues (ScalarE stays free for its PSUM->SBUF copies);
            # replica r goes to queue r so pieces complete in group order
            queues = (nc.sync, nc.gpsimd)
            for r, base in enumerate((0, 32)):
                queues[r].dma_start(lhs_sb[base:base + K, :], lhs_d[:])
            cut = max(2, G // 4)         # small first piece: compute starts early
            for p, q in ((0, cut), (cut, G)):
                for r, base in enumerate((0, 32)):
                    queues[r].dma_start(
                        rhs_sb[base:base + K, p * GROUP:q * GROUP],
                        rhs_d[:, p * GROUP:q * GROUP])

            for g in range(G):
                lhsT = {rg: lhs_sb[32 * rg:32 * rg + K, g * 128:(g + 1) * 128]
                        for rg in (0, 1)}
                ps = ppool.tile([128, GROUP], fp32, tag="ps")
                for j in range(GROUP // M_CHUNK):
                    rg = j % 2      # alternate row groups -> LDW overlaps MM
                    c = g * GROUP + j * M_CHUNK
                    nc.tensor.matmul(
                        ps[:, j * M_CHUNK:(j + 1) * M_CHUNK],
                        lhsT[rg],
                        rhs_sb[32 * rg:32 * rg + K, c:c + M_CHUNK],
                        start=True, stop=True)
                # only one DVE input may be PSUM: ScalarE stages the second half
                half = spool.tile([128, GROUP // 2], fp32, tag="half")
                nc.scalar.copy(half[:], ps[:, GROUP // 2:])
                scr = spool.tile([128, GROUP // 2], fp32, tag="scr")
                nc.vector._custom_dve(
                    min2,
                    out=scr[:],
                    in0=ps[:, :GROUP // 2],
                    in1=half[:],
                    s0=_BIG,
                    accum_out=acc[:, g:g + 1],
                )

            nc.sync.dma_start(out_d[:], acc[:])
    nc.finalize()
    return nc


def _get_nc(G):
    if G not in _NC_CACHE:
        _NC_CACHE[G] = _build_bass(G)
    return _NC_CACHE[G]


# ---------------------------------------------------------------- host planning
def _plan_batch(s, t):
    """Sort by radius, certify per-tile target chunk windows (exact)."""
    s = s.astype(np.float64)
    t = t.astype(np.float64)
    n, m = len(s), len(t)
    sn = np.linalg.norm(s, axis=1)
    tn = np.linalg.norm(t, axis=1)
    so = np.argsort(sn, kind="stable")
    to = np.argsort(tn, kind="stable")
    s_s, sn_s = s[so], sn[so]
    t_s, tn_s = t[to], tn[to]

    # upper bound on each source's NN distance from k rank-adjacent candidates
    idx = np.searchsorted(tn_s, sn_s)
    lo = np.clip(idx - K_CAND // 2, 0, m - K_CAND)
    cand_idx = lo[:, None] + np.arange(K_CAND)[None, :]
    d2 = ((s_s[:, None, :] - t_s[cand_idx]) ** 2).sum(-1)
    ub = d2.min(1)

    W = np.sqrt(ub)
    ntiles = n // 128
    windows = []
    for ti in range(ntiles):
        sl = slice(ti * 128, (ti + 1) * 128)
        Wt = W[sl].max() * (1 + 1e-9) + 1e-12
        lo_t = np.searchsorted(tn_s, sn_s[sl].min() - Wt, side="left")
        hi_t = np.searchsorted(tn_s, sn_s[sl].max() + Wt, side="right")
        lo_c = int(lo_t) // M_CHUNK
        hi_c = min((int(hi_t) + M_CHUNK - 1) // M_CHUNK, m // M_CHUNK)
        # round to an even number of chunks (GROUP = 2 chunks), stay in range
        nch = hi_c - lo_c
        if nch % 2:
            if hi_c < m // M_CHUNK:
                hi_c += 1
            elif lo_c > 0:
                lo_c -= 1
            else:
                hi_c += 1           # pad beyond end; gather clips (dup cols)
        windows.append((lo_c, hi_c))
    return s_s, t_s, sn_s, windows


def _prepare_inputs(source_point_cloud, target_point_cloud):
    s_all = np.asarray(source_point_cloud, dtype=np.float32)
    t_all = np.asarray(target_point_cloud, dtype=np.float32)

    # plan per batch
    plans = []
    for b in range(B):
        s_s, t_s, sn_s, windows = _plan_batch(s_all[b], t_all[b])
        # flat group list: (tile_idx, chunk_lo) per GROUP (=2 chunks)
        groups = []
        for ti, (lo_c, hi_c) in enumerate(windows):
            for c in range(lo_c, hi_c, 2):
                groups.append((ti, c))
        plans.append({"s": s_s, "t": t_s, "groups": groups})

    g_per_core = max((len(p["groups"]) + CORES_PER_BATCH - 1) // CORES_PER_BATCH
                     for p in plans)
    G = max(g_per_core, 2)

    # build per-batch operand pieces
    batch_data = []
    for b in range(B):
        p = plans[b]
        s_s, t_s = p["s"], p["t"]
        sh, sl, sl2 = _split3(s_s)
        s2 = (s_s ** 2).sum(-1)          # fp64
        s2h, s2l, s2l2 = _split3(s2)
        th, tl, tl2 = _split3(t_s)
        t2 = (t_s ** 2).sum(-1)
        t2h, t2l, t2l2 = _split3(t2)

        # K x n lhs rows and K x m rhs rows (sorted order)
        nn_ = len(s_s); mm_ = len(t_s)
        lhs_rows = np.zeros((K, nn_), dtype=bf16)
        rhs_rows = np.zeros((K, mm_), dtype=bf16)

        def m2(x):
            return (np.float32(-2.0) * x.astype(np.float32)).astype(bf16)

        for d in range(D):
            lhs_rows[0 + d] = sh[:, d];  rhs_rows[0 + d] = m2(th[:, d])
            lhs_rows[3 + d] = sh[:, d];  rhs_rows[3 + d] = m2(tl[:, d])
            lhs_rows[6 + d] = sl[:, d];  rhs_rows[6 + d] = m2(th[:, d])
            lhs_rows[9 + d] = sl[:, d];  rhs_rows[9 + d] = m2(tl[:, d])
            lhs_rows[12 + d] = sh[:, d]; rhs_rows[12 + d] = m2(tl2[:, d])
            lhs_rows[15 + d] = sl2[:, d]; rhs_rows[15 + d] = m2(th[:, d])
        one_n = np.ones(nn_, dtype=bf16); one_m = np.ones(mm_, dtype=bf16)
        lhs_rows[18] = one_n; rhs_rows[18] = t2h
        lhs_rows[19] = one_n; rhs_rows[19] = t2l
        lhs_rows[20] = one_n; rhs_rows[20] = t2l2
        lhs_rows[21] = s2h;   rhs_rows[21] = one_m
        lhs_rows[22] = s2l;   rhs_rows[22] = one_m
        lhs_rows[23] = s2l2;  rhs_rows[23] = one_m

        s2_dev = (s2h.astype(np.float64) + s2l.astype(np.float64)
                  + s2l2.astype(np.float64))
        batch_data.append({
            "lhs_rows": lhs_rows, "rhs_rows": rhs_rows,
            "s2_resid": s2 - s2_dev, "groups": plans[b]["groups"],
            "m_chunks": mm_ // M_CHUNK,
        })

    # assign contiguous slabs of the flat group list to cores; pad with
    # duplicates of group 0 (host ignores padded columns)
    in_maps, core_maps = [], []
    for core in range(N_CORES):
        b = core // CORES_PER_BATCH
        q = core % CORES_PER_BATCH
        bd = batch_data[b]
        groups = bd["groups"]
        per = (len(groups) + CORES_PER_BATCH - 1) // CORES_PER_BATCH
        sel = groups[q * per:(q + 1) * per]
        pad = G - len(sel)
        sel_padded = sel + [groups[0]] * pad if sel else [groups[0]] * G

        lhs = np.zeros((K, G * 128), dtype=bf16)
        rhs = np.zeros((K, G * GROUP), dtype=bf16)
        mc = bd["m_chunks"]
        for gi, (ti, c) in enumerate(sel_padded):
            lhs[:, gi * 128:(gi + 1) * 128] = \
                bd["lhs_rows"][:, ti * 128:(ti + 1) * 128]
            c2 = min(c + 2, mc)          # clip; duplicate last chunk if needed
            cols = bd["rhs_rows"][:, c * M_CHUNK:c2 * M_CHUNK]
            if c2 - c < 2:
                cols = np.concatenate([cols, cols[:, :M_CHUNK]], axis=1)
            rhs[:, gi * GROUP:(gi + 1) * GROUP] = cols

        in_maps.append({"lhs": lhs, "rhs": rhs})
        core_maps.append({"batch": b, "sel": sel, "n_real": len(sel)})

    return G, in_maps, core_maps, batch_data


def _run(source_point_cloud, target_point_cloud, trace=False):
    G, in_maps, core_maps, batch_data = _prepare_inputs(
        source_point_cloud, target_point_cloud)
    nc = _get_nc(G)
    res = run_bass_kernel_spmd(nc, in_maps, core_ids=list(range(N_CORES)),
                               trace=trace)

    # host combine: per batch, min over each tile's group columns
    ntiles = N // 128
    best = [np.full((ntiles * 128,), np.inf) for _ in range(B)]
    for core in range(N_CORES):
        cm = core_maps[core]
        out = res.results[core]["out"].astype(np.float64)  # [128, G]
        bb = best[cm["batch"]]
        for gi, (ti, _c) in enumerate(cm["sel"]):
            rows = slice(ti * 128, (ti + 1) * 128)
            bb[rows] = np.minimum(bb[rows], out[:, gi])
    total = 0.0
    for b in range(B):
        total += best[b].sum() + batch_data[b]["s2_resid"].sum()
    loss = total / (B * N * D)
    return np.float32(loss), res


def kernel(source_point_cloud, target_point_cloud):
    out, _ = _run(source_point_cloud, target_point_cloud,
                  trace=bool(os.environ.get("BASS_TRACE")))
    return out


# revision 19
# speedup vs baseline: 1.2780x; 1.2780x over previous
"""Trainium2 Bass kernel for KDPointToPointLoss (exact 1-NN + MSE).

Math: loss = mean_b mean_{n,d} ||s_n - t_{nn(n)}||^2
           = (1/(B*N*3)) * sum_{b,n} min_m ||s_n - t_m||^2
so only the min distance VALUES are needed (no argmin indices / gather).

Exact norm-window pruning: sort sources and targets by radius (the loss is
permutation invariant). For a source tile (128 radius-adjacent sources) with
radius range [a,b] and a certified upper bound W >= max_n sqrt(min-dist_n),
every nearest neighbor lies among targets with radius in [a-W, b+W]: any
other target m has d2 >= (|t_m|-|s_n|)^2 > W^2 >= min-dist. W comes from a
cheap host scan of k rank-adjacent candidates (valid upper bound; the device
still evaluates every certified candidate exactly). This prunes ~85% of the
distance matrix on random clouds.

Device work = flat list of groups (source tile x 1024 gathered target cols):
K=24 bf16 matmul (hi/lo/lo2 splits of s, t, s2, t2 -> fp32-level accuracy)
into PSUM, then a custom 2-input DVE op (min body + min accumulate,
2 elems/cycle) folds each group to one accumulator column. ScalarE stages
half of each group PSUM->SBUF (DVE may read only one PSUM operand).
Matmuls alternate two row-group weight replicas so LDWEIGHTS overlaps the
other group's in-flight matmul. Host min-combines group columns (fp64).

Sharding: 8 cores; cores 0-3 batch 0, cores 4-7 batch 1, balanced by group
count; the gathered rhs keeps per-core inputs small.
"""

import os
import numpy as np
import ml_dtypes

import concourse.bass as bass
import concourse.bacc as bacc
import concourse.mybir as mybir
from concourse.tile import TileContext
from concourse.bass_utils import run_bass_kernel_spmd

bf16 = ml_dtypes.bfloat16

B, N, M, D = 2, 8192, 8192, 3
N_CORES = 8
CORES_PER_BATCH = N_CORES // B
M_CHUNK = 512
GROUP = 1024                 # columns per DVE fold group (2 PSUM banks)
K = 24
K_CAND = 512                 # host candidate scan width for upper bounds
_BIG = 3.0e38

_DMA_SPLIT = 6               # rhs pieces per replica, spread over DMA queues


# ---------------------------------------------------------------- custom DVE op
_MIN2 = None


def _get_min2_op():
    """MIN2_REDUCE_ANT: out = min(in0, in1); accum = min(s0, min(out)).
    Reads 2 tensor streams at 1 elem/cycle each -> 2x native tensor_reduce."""
    global _MIN2
    if _MIN2 is not None:
        return _MIN2
    import concourse.dve_ops as dve_ops
    from concourse.dve_spec import Spec, Src0, Src1, C0, minn, lower, _has_src1
    from concourse.dve_uop import DveOpSpec

    for op in dve_ops.OPS:
        if op.name == "MIN2_REDUCE_ANT":
            _MIN2 = op
            return op

    def _ref(in0, in1, c0, c1, c2):
        b = np.minimum(in0.astype(np.float32), in1.astype(np.float32))
        acc = np.minimum(
            np.minimum.reduce(b.reshape(b.shape[0], -1), axis=-1, keepdims=True),
            np.asarray(c0, np.float32).reshape(-1, 1))
        return b, acc

    spec = Spec(body=minn(Src0, Src1), accum=minn, accum_init=C0, reference=_ref)
    opcode = dve_ops._CUSTOM_DVE_ROW_BASE + len(dve_ops.OPS)
    sha = {}
    for ver in ("v3", "v4"):
        uops = lower(spec, ver=ver)
        sha[ver] = DveOpSpec(name="MIN2_REDUCE_ANT", opcode=opcode, uops=uops,
                             rd1_en=_has_src1(spec)).sha(ver)
    op = dve_ops.DveOp("MIN2_REDUCE_ANT", spec, subdim=False, uops_sha=sha)
    dve_ops.OPS.append(op)
    dve_ops._SUB_OPCODE_FOR_NAME[op.name] = opcode
    _MIN2 = op
    return op


def _split3(x):
    """fp64 array -> (hi, lo, lo2) bf16 triple with residual ~2^-24."""
    x = x.astype(np.float64)
    h = x.astype(bf16)
    r = x - h.astype(np.float64)
    l = r.astype(bf16)
    r2 = r - l.astype(np.float64)
    l2 = r2.astype(bf16)
    return h, l, l2


# ---------------------------------------------------------------- device kernel
_NC_CACHE = {}


def _build_bass(G):
    """Flat loop over G groups: 2 matmuls -> PSUM [128,1024], ScalarE stages
    the second half to SBUF, custom DVE op folds to acc[:, g]."""
    min2 = _get_min2_op()
    nc = bacc.Bacc(trn_type="TRN2")
    # 4 row-group replicas packed into 128 partitions (bases 0/32/64/96):
    # group g < Gh uses row groups 0/1, g >= Gh uses 2/3 on the same columns.
    # Full-width DMA is ~8x faster than partition-narrow transfers.
    Gh = (G + 1) // 2
    lhs_d = nc.dram_tensor("lhs", [128, Gh * 128], mybir.dt.bfloat16, kind="ExternalInput")
    rhs_d = nc.dram_tensor("rhs", [128, Gh * GROUP], mybir.dt.bfloat16, kind="ExternalInput")
    out_d = nc.dram_tensor("out", [128, G], mybir.dt.float32, kind="ExternalOutput")

    fp32 = mybir.dt.float32

    with TileContext(nc) as tc:
        with (
            tc.tile_pool(name="const", bufs=1) as cpool,
            tc.tile_pool(name="psum", bufs=4, space="PSUM") as ppool,
            tc.tile_pool(name="scratch", bufs=4) as spool,
        ):
            lhs_sb = cpool.tile([128, Gh * 128], mybir.dt.bfloat16)
            rhs_sb = cpool.tile([128, Gh * GROUP], mybir.dt.bfloat16)
            acc = cpool.tile([128, G], fp32)

            nc.sync.dma_start(lhs_sb[:], lhs_d[:])
            # column pieces in consumption order; a piece serves group g AND
            # its partner g+Gh (different row groups, same columns)
            cuts = [0, min(2, Gh), min(5, Gh), Gh]
            for p, q in zip(cuts, cuts[1:]):
                if q > p:
                    nc.sync.dma_start(rhs_sb[:, p * GROUP:q * GROUP],
                                      rhs_d[:, p * GROUP:q * GROUP])

            for g in range(G):
                half2 = g >= Gh
                gc = g - Gh if half2 else g          # column slot
                rgs = (2, 3) if half2 else (0, 1)
                ps = ppool.tile([128, GROUP], fp32, tag="ps")
                for j in range(GROUP // M_CHUNK):
                    rg = rgs[j % 2]  # alternate row groups -> LDW overlaps MM
                    c = gc * GROUP + j * M_CHUNK
                    nc.tensor.matmul(
                        ps[:, j * M_CHUNK:(j + 1) * M_CHUNK],
                        lhs_sb[32 * rg:32 * rg + K, gc * 128:(gc + 1) * 128],
                        rhs_sb[32 * rg:32 * rg + K, c:c + M_CHUNK],
                        start=True, stop=True,
                        tile_position=(32 * rg, 0))
                # only one DVE input may be PSUM: ScalarE stages the second half
                half = spool.tile([128, GROUP // 2], fp32, tag="half")
                nc.scalar.copy(half[:], ps[:, GROUP // 2:])
                scr = spool.tile([128, GROUP // 2], fp32, tag="scr")
                nc.vector._custom_dve(
                    min2,
                    out=scr[:],
                    in0=ps[:, :GROUP // 2],
                    in1=half[:],
                    s0=_BIG,
                    accum_out=acc[:, g:g + 1],
                )

            nc.sync.dma_start(out_d[:], acc[:])
    nc.finalize()
    return nc


def _get_nc(G):
    if G not in _NC_CACHE:
        _NC_CACHE[G] = _build_bass(G)
    return _NC_CACHE[G]


# ---------------------------------------------------------------- host planning
def _plan_batch(s, t):
    """Sort by radius, certify per-tile target chunk windows (exact)."""
    s = s.astype(np.float64)
    t = t.astype(np.float64)
    n, m = len(s), len(t)
    sn = np.linalg.norm(s, axis=1)
    tn = np.linalg.norm(t, axis=1)
    so = np.argsort(sn, kind="stable")
    to = np.argsort(tn, kind="stable")
    s_s, sn_s = s[so], sn[so]
    t_s, tn_s = t[to], tn[to]

    # upper bound on each source's NN distance from k rank-adjacent candidates
    idx = np.searchsorted(tn_s, sn_s)
    lo = np.clip(idx - K_CAND // 2, 0, m - K_CAND)
    cand_idx = lo[:, None] + np.arange(K_CAND)[None, :]
    d2 = ((s_s[:, None, :] - t_s[cand_idx]) ** 2).sum(-1)
    ub = d2.min(1)

    W = np.sqrt(ub)
    ntiles = n // 128
    windows = []
    for ti in range(ntiles):
        sl = slice(ti * 128, (ti + 1) * 128)
        Wt = W[sl].max() * (1 + 1e-9) + 1e-12
        lo_t = np.searchsorted(tn_s, sn_s[sl].min() - Wt, side="left")
        hi_t = np.searchsorted(tn_s, sn_s[sl].max() + Wt, side="right")
        lo_c = int(lo_t) // M_CHUNK
        hi_c = min((int(hi_t) + M_CHUNK - 1) // M_CHUNK, m // M_CHUNK)
        # round to an even number of chunks (GROUP = 2 chunks), stay in range
        nch = hi_c - lo_c
        if nch % 2:
            if hi_c < m // M_CHUNK:
                hi_c += 1
            elif lo_c > 0:
                lo_c -= 1
            else:
                hi_c += 1           # pad beyond end; gather clips (dup cols)
        windows.append((lo_c, hi_c))
    return s_s, t_s, sn_s, windows


def _prepare_inputs(source_point_cloud, target_point_cloud):
    s_all = np.asarray(source_point_cloud, dtype=np.float32)
    t_all = np.asarray(target_point_cloud, dtype=np.float32)

    # plan per batch
    plans = []
    for b in range(B):
        s_s, t_s, sn_s, windows = _plan_batch(s_all[b], t_all[b])
        # flat group list: (tile_idx, chunk_lo) per GROUP (=2 chunks)
        groups = []
        for ti, (lo_c, hi_c) in enumerate(windows):
            for c in range(lo_c, hi_c, 2):
                groups.append((ti, c))
        plans.append({"s": s_s, "t": t_s, "groups": groups})

    g_per_core = max((len(p["groups"]) + CORES_PER_BATCH - 1) // CORES_PER_BATCH
                     for p in plans)
    G = max(g_per_core, 2)

    # build per-batch operand pieces
    batch_data = []
    for b in range(B):
        p = plans[b]
        s_s, t_s = p["s"], p["t"]
        sh, sl, sl2 = _split3(s_s)
        s2 = (s_s ** 2).sum(-1)          # fp64
        s2h, s2l, s2l2 = _split3(s2)
        th, tl, tl2 = _split3(t_s)
        t2 = (t_s ** 2).sum(-1)
        t2h, t2l, t2l2 = _split3(t2)

        # K x n lhs rows and K x m rhs rows (sorted order)
        nn_ = len(s_s); mm_ = len(t_s)
        lhs_rows = np.zeros((K, nn_), dtype=bf16)
        rhs_rows = np.zeros((K, mm_), dtype=bf16)

        def m2(x):
            return (np.float32(-2.0) * x.astype(np.float32)).astype(bf16)

        for d in range(D):
            lhs_rows[0 + d] = sh[:, d];  rhs_rows[0 + d] = m2(th[:, d])
            lhs_rows[3 + d] = sh[:, d];  rhs_rows[3 + d] = m2(tl[:, d])
            lhs_rows[6 + d] = sl[:, d];  rhs_rows[6 + d] = m2(th[:, d])
            lhs_rows[9 + d] = sl[:, d];  rhs_rows[9 + d] = m2(tl[:, d])
            lhs_rows[12 + d] = sh[:, d]; rhs_rows[12 + d] = m2(tl2[:, d])
            lhs_rows[15 + d] = sl2[:, d]; rhs_rows[15 + d] = m2(th[:, d])
        one_n = np.ones(nn_, dtype=bf16); one_m = np.ones(mm_, dtype=bf16)
        lhs_rows[18] = one_n; rhs_rows[18] = t2h
        lhs_rows[19] = one_n; rhs_rows[19] = t2l
        lhs_rows[20] = one_n; rhs_rows[20] = t2l2
        lhs_rows[21] = s2h;   rhs_rows[21] = one_m
        lhs_rows[22] = s2l;   rhs_rows[22] = one_m
        lhs_rows[23] = s2l2;  rhs_rows[23] = one_m

        s2_dev = (s2h.astype(np.float64) + s2l.astype(np.float64)
                  + s2l2.astype(np.float64))
        batch_data.append({
            "lhs_rows": lhs_rows, "rhs_rows": rhs_rows,
            "s2_resid": s2 - s2_dev, "groups": plans[b]["groups"],
            "m_chunks": mm_ // M_CHUNK,
        })

    # assign contiguous slabs of the flat group list to cores; pad with
    # duplicates of group 0 (host ignores padded columns)
    in_maps, core_maps = [], []
    for core in range(N_CORES):
        b = core // CORES_PER_BATCH
        q = core % CORES_PER_BATCH
        bd = batch_data[b]
        groups = bd["groups"]
        per = (len(groups) + CORES_PER_BATCH - 1) // CORES_PER_BATCH
        sel = groups[q * per:(q + 1) * per]
        pad = G - len(sel)
        sel_padded = sel + [groups[0]] * pad if sel else [groups[0]] * G

        Gh = (G + 1) // 2
        lhs = np.zeros((128, Gh * 128), dtype=bf16)
        rhs = np.zeros((128, Gh * GROUP), dtype=bf16)
        mc = bd["m_chunks"]
        for gi, (ti, c) in enumerate(sel_padded):
            half2 = gi >= Gh
            gc = gi - Gh if half2 else gi
            bases = (64, 96) if half2 else (0, 32)
            ltile = bd["lhs_rows"][:, ti * 128:(ti + 1) * 128]
            c2 = min(c + 2, mc)          # clip; duplicate last chunk if needed
            cols = bd["rhs_rows"][:, c * M_CHUNK:c2 * M_CHUNK]
            if c2 - c < 2:
                cols = np.concatenate([cols, cols[:, :M_CHUNK]], axis=1)
            for base in bases:
                lhs[base:base + K, gc * 128:(gc + 1) * 128] = ltile
                rhs[base:base + K, gc * GROUP:(gc + 1) * GROUP] = cols

        in_maps.append({"lhs": lhs, "rhs": rhs})
        core_maps.append({"batch": b, "sel": sel, "n_real": len(sel)})

    return G, in_maps, core_maps, batch_data


def _run(source_point_cloud, target_point_cloud, trace=False):
    G, in_maps, core_maps, batch_data = _prepare_inputs(
        source_point_cloud, target_point_cloud)
    nc = _get_nc(G)
    res = run_bass_kernel_spmd(nc, in_maps, core_ids=list(range(N_CORES)),
                               trace=trace)

    # host combine: per batch, min over each tile's group columns
    ntiles = N // 128
    best = [np.full((ntiles * 128,), np.inf) for _ in range(B)]
    for core in range(N_CORES):
        cm = core_maps[core]
        out = res.results[core]["out"].astype(np.float64)  # [128, G]
        bb = best[cm["batch"]]
        for gi, (ti, _c) in enumerate(cm["sel"]):
            rows = slice(ti * 128, (ti + 1) * 128)
            bb[rows] = np.minimum(bb[rows], out[:, gi])
    total = 0.0
    for b in range(B):
        total += best[b].sum() + batch_data[b]["s2_resid"].sum()
    loss = total / (B * N * D)
    return np.float32(loss), res


def kernel(source_point_cloud, target_point_cloud):
    out, _ = _run(source_point_cloud, target_point_cloud,
                  trace=bool(os.environ.get("BASS_TRACE")))
    return out
